# revision 3
# baseline (speedup 1.0000x reference)
"""Trainium2 Bass kernel for full-embed-dim self-attention + residual LayerNorm.

Problem: B=4, S=2048, D=1024 fp32.
  q/k/v = x@w{q,k,v}+b; scores = q@k^T/sqrt(D); attn = softmax(scores)@v;
  out = LN(x + attn@wo + bo) * gamma + beta.

Sharding: 8 cores = 4 batches x 2 query-halves (1024 queries each). Each
core recomputes K/V for its batch's full 2048-key sequence (cheaper than
exchanging them between the two cores of a batch).

Per-core dataflow (all in d-on-partitions "transposed" layout):
  QT[d,q]   = wq^T @ xq^T                      (from host-transposed x)
  KT[d,k]   = wk^T @ x^T  (+bk per-partition)
  V[k,d]    = x^T^T @ wv                       (per k-slab of 512)
  ST[k,q]   = KT^T @ QT        (contracted over d in PSUM)
  PT[k,q]   = exp(ST/sqrt(D) + kbias[k])       kbias = K@(bq/sqrt(D))
              (per-q softmax factors cancel after normalization; logits are
               O(5) std-1 so exp needs no max-subtraction in fp32)
  denom[q]  = ones^T @ PT      (PE column-sum)
  AT[d,q]   = V^T @ PT
  O[q,e]    = AT^T @ wo, then O/denom[q] + xq_aug (host adds bo+bv@wo), LN.

Matmul inputs are bf16 (1 PE cycle/row vs 4 for fp32; fp32 PSUM accum).
"""

import numpy as np
import ml_dtypes

import concourse.bass as bass
import concourse.mybir as mybir
import concourse.tile as tile
from concourse import bacc

F32 = mybir.dt.float32
BF16 = mybir.dt.bfloat16

B, S, D = 4, 2048, 1024
Q = 1024          # queries per core
SCALE = 1.0 / 32.0
EPS = 1e-6
NKC = S // 128    # 16 key chunks
NDC = D // 128    # 8 d chunks
SLAB = 512        # keys per phase-A slab
NSLAB = S // SLAB


def _bcast_ap(ap_1d, parts=128):
    """[N] dram AP -> [parts, N] AP with 0-stride partition dim."""
    return bass.AP(
        tensor=ap_1d.tensor, offset=ap_1d.offset, ap=[[0, parts]] + list(ap_1d.ap)
    )


def build_nc():
    nc = bacc.Bacc("TRN2", target_bir_lowering=False, debug=False, num_devices=8)

    xt = nc.dram_tensor("xt", [D, S], BF16, kind="ExternalInput")
    xqt = nc.dram_tensor("xqt", [D, Q], BF16, kind="ExternalInput")
    xq = nc.dram_tensor("xq", [Q, D], F32, kind="ExternalInput")
    wq_d = nc.dram_tensor("wq", [D, D], BF16, kind="ExternalInput")
    wk_d = nc.dram_tensor("wk", [D, D], BF16, kind="ExternalInput")
    wv_d = nc.dram_tensor("wv", [D, D], BF16, kind="ExternalInput")
    wo_d = nc.dram_tensor("wo", [D, D], BF16, kind="ExternalInput")
    bqT_d = nc.dram_tensor("bqT", [128, NDC], BF16, kind="ExternalInput")
    bkT_d = nc.dram_tensor("bkT", [128, NDC], F32, kind="ExternalInput")
    gamma_d = nc.dram_tensor("gamma", [D], F32, kind="ExternalInput")
    beta_d = nc.dram_tensor("beta", [D], F32, kind="ExternalInput")
    out_d = nc.dram_tensor("out", [Q, D], F32, kind="ExternalOutput")

    with tile.TileContext(nc) as tc:
        with (
            tc.tile_pool(name="small", bufs=1) as p_small,
            tc.tile_pool(name="wkv", bufs=2 * NDC) as p_wkv,
            tc.tile_pool(name="vsb", bufs=NKC) as p_v,
            tc.tile_pool(name="ptsb", bufs=NKC) as p_pt,
            tc.tile_pool(name="qtsb", bufs=NDC) as p_qt,
            tc.tile_pool(name="xts", bufs=NDC) as p_xt,
            tc.tile_pool(name="kts", bufs=NDC) as p_kt,
            tc.tile_pool(name="ps", bufs=5, space="PSUM") as p_ps,
            tc.tile_pool(name="ps1", bufs=2, space="PSUM") as p_ps1,
        ):
            # ---- constants / small tiles ----
            bqT = p_small.tile([128, NDC], BF16)
            nc.sync.dma_start(out=bqT[:, :], in_=bqT_d[:, :])
            bkT = p_small.tile([128, NDC], F32)
            nc.sync.dma_start(out=bkT[:, :], in_=bkT_d[:, :])
            ones = p_small.tile([128, 1], BF16)
            nc.vector.memset(ones[:, :], 1.0)
            eps_t = p_small.tile([128, 1], F32)
            nc.vector.memset(eps_t[:, :], EPS)
            kbias = p_small.tile([128, NKC], F32)
            recip = p_small.tile([128, 8], F32)

            # ---- load wk, wv (resident through attention) ----
            wk = [p_wkv.tile([128, D], BF16, tag="wkv", name=f"wk{i}") for i in range(NDC)]
            wv = [p_wkv.tile([128, D], BF16, tag="wkv", name=f"wv{i}") for i in range(NDC)]
            for dc in range(NDC):
                nc.sync.dma_start(out=wk[dc][:, :], in_=wk_d[128 * dc:128 * (dc + 1), :])
                nc.sync.dma_start(out=wv[dc][:, :], in_=wv_d[128 * dc:128 * (dc + 1), :])

            qt = [p_qt.tile([128, Q], BF16, tag="qt", name=f"qt{i}") for i in range(NDC)]
            vsb = [p_v.tile([128, D], BF16, tag="v", name=f"vsb{i}") for i in range(NKC)]
            pt = [p_pt.tile([128, Q], BF16, tag="pt", name=f"pt{i}") for i in range(NKC)]

            # ---- QT = wq^T @ xqT (no bias; bq enters via kbias) ----
            with (
                tc.tile_pool(name="wqp", bufs=NDC) as p_wq,
                tc.tile_pool(name="xqtp", bufs=NDC) as p_xqt,
            ):
                wq = [p_wq.tile([128, D], BF16, tag="wq", name=f"wq{i}") for i in range(NDC)]
                xqts = [p_xqt.tile([128, Q], BF16, tag="xqt", name=f"xqts{i}") for i in range(NDC)]
                for dc in range(NDC):
                    nc.sync.dma_start(out=wq[dc][:, :], in_=wq_d[128 * dc:128 * (dc + 1), :])
                    nc.sync.dma_start(out=xqts[dc][:, :], in_=xqt[128 * dc:128 * (dc + 1), :])
                for do in range(NDC):
                    for qh in range(2):
                        ps = p_ps.tile([128, 512], F32, tag="ps")
                        for di in range(NDC):
                            nc.tensor.matmul(
                                ps[:, :],
                                wq[di][:, 128 * do:128 * (do + 1)],
                                xqts[di][:, 512 * qh:512 * (qh + 1)],
                                start=(di == 0), stop=(di == NDC - 1),
                            )
                        nc.vector.tensor_copy(qt[do][:, 512 * qh:512 * (qh + 1)], ps[:, :])

            # ---- per-slab: KT slab, V slab, kbias, ST -> exp -> PT ----
            for sl in range(NSLAB):
                s0 = SLAB * sl
                xts = [p_xt.tile([128, SLAB], BF16, tag="xt", name=f"xts{sl}_{i}") for i in range(NDC)]
                for dc in range(NDC):
                    nc.sync.dma_start(
                        out=xts[dc][:, :], in_=xt[128 * dc:128 * (dc + 1), s0:s0 + SLAB]
                    )
                # KT slab [d, SLAB] (+bk, per-partition)
                kts = [p_kt.tile([128, SLAB], BF16, tag="kt", name=f"kts{sl}_{i}") for i in range(NDC)]
                for do in range(NDC):
                    ps = p_ps.tile([128, 512], F32, tag="ps")
                    for di in range(NDC):
                        nc.tensor.matmul(
                            ps[:, :],
                            wk[di][:, 128 * do:128 * (do + 1)],
                            xts[di][:, :],
                            start=(di == 0), stop=(di == NDC - 1),
                        )
                    nc.vector.tensor_scalar(
                        out=kts[do][:, :], in0=ps[:, :],
                        scalar1=bkT[:, do:do + 1], scalar2=None,
                        op0=mybir.AluOpType.add,
                    )
                # V slab rows [k, D]
                for kl in range(SLAB // 128):
                    kc = (s0 // 128) + kl
                    for dh in range(2):
                        ps = p_ps.tile([128, 512], F32, tag="ps")
                        for di in range(NDC):
                            nc.tensor.matmul(
                                ps[:, :],
                                xts[di][:, 128 * kl:128 * (kl + 1)],
                                wv[di][:, 512 * dh:512 * (dh + 1)],
                                start=(di == 0), stop=(di == NDC - 1),
                            )
                        nc.vector.tensor_copy(vsb[kc][:, 512 * dh:512 * (dh + 1)], ps[:, :])
                # kbias[k] = K @ (bq/sqrt(D)) for this slab
                for kl in range(SLAB // 128):
                    kc = (s0 // 128) + kl
                    ps1 = p_ps1.tile([128, 1], F32, tag="ps1")
                    for do in range(NDC):
                        nc.tensor.matmul(
                            ps1[:, :],
                            kts[do][:, 128 * kl:128 * (kl + 1)],
                            bqT[:, do:do + 1],
                            start=(do == 0), stop=(do == NDC - 1),
                        )
                    nc.vector.tensor_copy(kbias[:, kc:kc + 1], ps1[:, :])
                # ST -> exp -> PT
                for kl in range(SLAB // 128):
                    kc = (s0 // 128) + kl
                    for qh in range(2):
                        ps = p_ps.tile([128, 512], F32, tag="ps")
                        for do in range(NDC):
                            nc.tensor.matmul(
                                ps[:, :],
                                kts[do][:, 128 * kl:128 * (kl + 1)],
                                qt[do][:, 512 * qh:512 * (qh + 1)],
                                start=(do == 0), stop=(do == NDC - 1),
                            )
                        nc.scalar.activation(
                            out=pt[kc][:, 512 * qh:512 * (qh + 1)], in_=ps[:, :],
                            func=mybir.ActivationFunctionType.Exp,
                            bias=kbias[:, kc:kc + 1], scale=SCALE,
                        )

            # ---- AT[d,q] = V^T @ PT ; denom[q] = ones^T @ PT ----
            with tc.tile_pool(name="atp", bufs=NDC) as p_at:
                at = [p_at.tile([128, Q], BF16, tag="at", name=f"at{i}") for i in range(NDC)]
                for dc in range(NDC):
                    for qh in range(2):
                        ps = p_ps.tile([128, 512], F32, tag="ps")
                        for kc in range(NKC):
                            nc.tensor.matmul(
                                ps[:, :],
                                vsb[kc][:, 128 * dc:128 * (dc + 1)],
                                pt[kc][:, 512 * qh:512 * (qh + 1)],
                                start=(kc == 0), stop=(kc == NKC - 1),
                            )
                        nc.vector.tensor_copy(at[dc][:, 512 * qh:512 * (qh + 1)], ps[:, :])
                for qp in range(8):
                    ps1 = p_ps1.tile([128, 1], F32, tag="ps1")
                    for kc in range(NKC):
                        nc.tensor.matmul(
                            ps1[:, :],
                            pt[kc][:, 128 * qp:128 * (qp + 1)],
                            ones[:, :],
                            start=(kc == 0), stop=(kc == NKC - 1),
                        )
                    nc.vector.reciprocal(recip[:, qp:qp + 1], ps1[:, :])

                # ---- O = AT^T @ wo ; /denom ; +xq_aug ; LayerNorm ----
                with (
                    tc.tile_pool(name="wop", bufs=NDC) as p_wo,
                    tc.tile_pool(name="xqp", bufs=3) as p_xq,
                    tc.tile_pool(name="vout", bufs=3) as p_vo,
                    tc.tile_pool(name="lnst", bufs=4) as p_ln,
                ):
                    wo = [p_wo.tile([128, D], BF16, tag="wo", name=f"wo{i}") for i in range(NDC)]
                    for dc in range(NDC):
                        nc.sync.dma_start(out=wo[dc][:, :], in_=wo_d[128 * dc:128 * (dc + 1), :])
                    gam = p_small.tile([128, D], F32)
                    nc.gpsimd.dma_start(out=gam[:, :], in_=_bcast_ap(gamma_d[:]))
                    bet = p_small.tile([128, D], F32)
                    nc.gpsimd.dma_start(out=bet[:, :], in_=_bcast_ap(beta_d[:]))

                    for qp in range(8):
                        v = p_vo.tile([128, D], F32, tag="v")
                        xqt_ = p_xq.tile([128, D], F32, tag="xq")
                        nc.sync.dma_start(
                            out=xqt_[:, :], in_=xq[128 * qp:128 * (qp + 1), :]
                        )
                        for eh in range(2):
                            ps = p_ps.tile([128, 512], F32, tag="ps")
                            for dc in range(NDC):
                                nc.tensor.matmul(
                                    ps[:, :],
                                    at[dc][:, 128 * qp:128 * (qp + 1)],
                                    wo[dc][:, 512 * eh:512 * (eh + 1)],
                                    start=(dc == 0), stop=(dc == NDC - 1),
                                )
                            nc.vector.tensor_scalar(
                                out=v[:, 512 * eh:512 * (eh + 1)], in0=ps[:, :],
                                scalar1=recip[:, qp:qp + 1], scalar2=None,
                                op0=mybir.AluOpType.mult,
                            )
                        nc.vector.tensor_add(v[:, :], v[:, :], xqt_[:, :])
                        stats = p_ln.tile([128, 2, 6], F32, tag="st")
                        vg = v[:, :].rearrange("p (g d) -> p g d", g=2)
                        for g in range(2):
                            nc.vector.bn_stats(out=stats[:, g, :], in_=vg[:, g, :])
                        mv = p_ln.tile([128, 2], F32, tag="mv")
                        nc.vector.bn_aggr(out=mv[:, :], in_=stats[:, :])
                        nc.scalar.activation(
                            out=mv[:, 1:2], in_=mv[:, 1:2],
                            func=mybir.ActivationFunctionType.Sqrt,
                            bias=eps_t[:, :],
                        )
                        nc.vector.reciprocal(mv[:, 1:2], mv[:, 1:2])
                        nc.vector.tensor_scalar(
                            out=v[:, :], in0=v[:, :],
                            scalar1=mv[:, 0:1], scalar2=mv[:, 1:2],
                            op0=mybir.AluOpType.subtract, op1=mybir.AluOpType.mult,
                        )
                        nc.vector.tensor_mul(v[:, :], v[:, :], gam[:, :])
                        nc.vector.tensor_add(v[:, :], v[:, :], bet[:, :])
                        nc.sync.dma_start(out=out_d[128 * qp:128 * (qp + 1), :], in_=v[:, :])
    nc.compile()
    return nc


_NC_CACHE = None


def kernel(**inputs) -> np.ndarray:
    from concourse.bass_utils import run_bass_kernel_spmd

    global _NC_CACHE
    x = np.asarray(inputs["inputs"], np.float32)
    wq = np.asarray(inputs["wq"], np.float32)
    wk = np.asarray(inputs["wk"], np.float32)
    wv = np.asarray(inputs["wv"], np.float32)
    wo = np.asarray(inputs["wo"], np.float32)
    bq = np.asarray(inputs["bq"], np.float32)
    bk = np.asarray(inputs["bk"], np.float32)
    bv = np.asarray(inputs["bv"], np.float32)
    bo = np.asarray(inputs["bo"], np.float32)
    gamma = np.asarray(inputs["gamma"], np.float32)
    beta = np.asarray(inputs["beta"], np.float32)

    bf = lambda a: np.ascontiguousarray(a).astype(ml_dtypes.bfloat16)
    bo_eff = bo + bv @ wo  # exact: rows of softmax sum to 1
    shared = {
        "wq": bf(wq), "wk": bf(wk), "wv": bf(wv), "wo": bf(wo),
        "bqT": bf((bq * SCALE).reshape(NDC, 128).T),
        "bkT": np.ascontiguousarray(bk.reshape(NDC, 128).T),
        "gamma": gamma, "beta": beta,
    }
    in_maps = []
    for c in range(8):
        b, qh = c // 2, c % 2
        xtb = np.ascontiguousarray(x[b].T)
        in_maps.append({
            **shared,
            "xt": bf(xtb),
            "xqt": bf(xtb[:, Q * qh:Q * (qh + 1)]),
            "xq": np.ascontiguousarray(x[b, Q * qh:Q * (qh + 1), :]) + bo_eff[None, :],
        })

    if _NC_CACHE is None:
        _NC_CACHE = build_nc()
    res = run_bass_kernel_spmd(_NC_CACHE, in_maps, core_ids=list(range(8)))
    out = np.empty((B, S, D), np.float32)
    for c in range(8):
        b, qh = c // 2, c % 2
        out[b, Q * qh:Q * (qh + 1), :] = res.results[c]["out"]
    return out


# revision 4
# speedup vs baseline: 1.0454x; 1.0454x over previous
"""Trainium2 Bass kernel for full-embed-dim self-attention + residual LayerNorm.

Problem: B=4, S=2048, D=1024 fp32.
  q/k/v = x@w{q,k,v}+b; scores = q@k^T/sqrt(D); attn = softmax(scores)@v;
  out = LN(x + attn@wo + bo) * gamma + beta.

Sharding: 8 cores = 4 batches x 2 query-halves (1024 queries each). Each
core recomputes K/V for its batch's full 2048-key sequence (cheaper than
exchanging them between the two cores of a batch).

Per-core dataflow (all in d-on-partitions "transposed" layout):
  QT[d,q]   = wq^T @ xq^T                      (from host-transposed x)
  KT[d,k]   = wk^T @ x^T  (+bk per-partition)
  V[k,d]    = x^T^T @ wv                       (per k-slab of 512)
  ST[k,q]   = KT^T @ QT        (contracted over d in PSUM)
  PT[k,q]   = exp(ST/sqrt(D) + kbias[k])       kbias = K@(bq/sqrt(D))
              (per-q softmax factors cancel after normalization; logits are
               O(5) std-1 so exp needs no max-subtraction in fp32)
  denom[q]  = ones^T @ PT      (PE column-sum)
  AT[d,q]   = V^T @ PT
  O[q,e]    = AT^T @ wo, then O/denom[q] + xq_aug (host adds bo+bv@wo), LN.

Matmul inputs are bf16 (1 PE cycle/row vs 4 for fp32; fp32 PSUM accum).
"""

import numpy as np
import ml_dtypes

import concourse.bass as bass
import concourse.mybir as mybir
import concourse.tile as tile
from concourse import bacc

F32 = mybir.dt.float32
BF16 = mybir.dt.bfloat16

B, S, D = 4, 2048, 1024
Q = 1024          # queries per core
SCALE = 1.0 / 32.0
EPS = 1e-6
NKC = S // 128    # 16 key chunks
NDC = D // 128    # 8 d chunks
SLAB = 512        # keys per phase-A slab
NSLAB = S // SLAB


def _bcast_ap(ap_1d, parts=128):
    """[N] dram AP -> [parts, N] AP with 0-stride partition dim."""
    return bass.AP(
        tensor=ap_1d.tensor, offset=ap_1d.offset, ap=[[0, parts]] + list(ap_1d.ap)
    )


def build_nc():
    nc = bacc.Bacc("TRN2", target_bir_lowering=False, debug=False, num_devices=8)

    xt = nc.dram_tensor("xt", [D, S], BF16, kind="ExternalInput")
    xqt = nc.dram_tensor("xqt", [D, Q], BF16, kind="ExternalInput")
    xq = nc.dram_tensor("xq", [Q, D], F32, kind="ExternalInput")
    wq_d = nc.dram_tensor("wq", [D, D], BF16, kind="ExternalInput")
    wk_d = nc.dram_tensor("wk", [D, D], BF16, kind="ExternalInput")
    wv_d = nc.dram_tensor("wv", [D, D], BF16, kind="ExternalInput")
    wo_d = nc.dram_tensor("wo", [D, D], BF16, kind="ExternalInput")
    bqT_d = nc.dram_tensor("bqT", [128, NDC], BF16, kind="ExternalInput")
    bkT_d = nc.dram_tensor("bkT", [128, NDC], F32, kind="ExternalInput")
    gamma_d = nc.dram_tensor("gamma", [D], F32, kind="ExternalInput")
    beta_d = nc.dram_tensor("beta", [D], F32, kind="ExternalInput")
    out_d = nc.dram_tensor("out", [Q, D], F32, kind="ExternalOutput")

    with tile.TileContext(nc) as tc:
        with (
            tc.tile_pool(name="small", bufs=1) as p_small,
            tc.tile_pool(name="wkv", bufs=2 * NDC) as p_wkv,
            tc.tile_pool(name="vsb", bufs=NKC) as p_v,
            tc.tile_pool(name="ptsb", bufs=NKC) as p_pt,
            tc.tile_pool(name="qtsb", bufs=NDC) as p_qt,
            tc.tile_pool(name="xts", bufs=NDC) as p_xt,
            tc.tile_pool(name="kts", bufs=NDC) as p_kt,
            tc.tile_pool(name="ps", bufs=6, space="PSUM") as p_ps,
            tc.tile_pool(name="ps1", bufs=2, space="PSUM") as p_ps1,
        ):
            # ---- constants / small tiles ----
            bqT = p_small.tile([128, NDC], BF16)
            nc.gpsimd.dma_start(out=bqT[:, :], in_=bqT_d[:, :])
            bkT = p_small.tile([128, NDC], F32)
            nc.gpsimd.dma_start(out=bkT[:, :], in_=bkT_d[:, :])
            ones = p_small.tile([128, 1], BF16)
            nc.vector.memset(ones[:, :], 1.0)
            eps_t = p_small.tile([128, 1], F32)
            nc.vector.memset(eps_t[:, :], EPS)
            kbias = p_small.tile([128, NKC], F32)
            recip = p_small.tile([128, 8], F32)

            # ---- load wk, wv (resident through attention) ----
            wk = [p_wkv.tile([128, D], BF16, tag="wkv", name=f"wk{i}") for i in range(NDC)]
            wv = [p_wkv.tile([128, D], BF16, tag="wkv", name=f"wv{i}") for i in range(NDC)]
            for dc in range(NDC):
                nc.gpsimd.dma_start(out=wk[dc][:, :], in_=wk_d[128 * dc:128 * (dc + 1), :])
                nc.gpsimd.dma_start(out=wv[dc][:, :], in_=wv_d[128 * dc:128 * (dc + 1), :])

            qt = [p_qt.tile([128, Q], BF16, tag="qt", name=f"qt{i}") for i in range(NDC)]
            vsb = [p_v.tile([128, D], BF16, tag="v", name=f"vsb{i}") for i in range(NKC)]
            pt = [p_pt.tile([128, Q], BF16, tag="pt", name=f"pt{i}") for i in range(NKC)]

            # ---- QT = wq^T @ xqT (no bias; bq enters via kbias) ----
            with (
                tc.tile_pool(name="wqp", bufs=NDC) as p_wq,
                tc.tile_pool(name="xqtp", bufs=NDC) as p_xqt,
            ):
                wq = [p_wq.tile([128, D], BF16, tag="wq", name=f"wq{i}") for i in range(NDC)]
                xqts = [p_xqt.tile([128, Q], BF16, tag="xqt", name=f"xqts{i}") for i in range(NDC)]
                for dc in range(NDC):
                    nc.sync.dma_start(out=wq[dc][:, :], in_=wq_d[128 * dc:128 * (dc + 1), :])
                    nc.sync.dma_start(out=xqts[dc][:, :], in_=xqt[128 * dc:128 * (dc + 1), :])
                for do in range(NDC):
                    for qh in range(2):
                        ps = p_ps.tile([128, 512], F32, tag="ps")
                        for di in range(NDC):
                            nc.tensor.matmul(
                                ps[:, :],
                                wq[di][:, 128 * do:128 * (do + 1)],
                                xqts[di][:, 512 * qh:512 * (qh + 1)],
                                start=(di == 0), stop=(di == NDC - 1),
                            )
                        nc.vector.tensor_copy(qt[do][:, 512 * qh:512 * (qh + 1)], ps[:, :])

            # ---- per-slab: KT slab, V slab, kbias, ST -> exp -> PT ----
            for sl in range(NSLAB):
                s0 = SLAB * sl
                xts = [p_xt.tile([128, SLAB], BF16, tag="xt", name=f"xts{sl}_{i}") for i in range(NDC)]
                for dc in range(NDC):
                    nc.sync.dma_start(
                        out=xts[dc][:, :], in_=xt[128 * dc:128 * (dc + 1), s0:s0 + SLAB]
                    )
                # KT slab [d, SLAB] (+bk, per-partition)
                kts = [p_kt.tile([128, SLAB], BF16, tag="kt", name=f"kts{sl}_{i}") for i in range(NDC)]
                for do in range(NDC):
                    ps = p_ps.tile([128, 512], F32, tag="ps")
                    for di in range(NDC):
                        nc.tensor.matmul(
                            ps[:, :],
                            wk[di][:, 128 * do:128 * (do + 1)],
                            xts[di][:, :],
                            start=(di == 0), stop=(di == NDC - 1),
                        )
                    nc.vector.tensor_scalar(
                        out=kts[do][:, :], in0=ps[:, :],
                        scalar1=bkT[:, do:do + 1], scalar2=None,
                        op0=mybir.AluOpType.add,
                    )
                # V slab rows [k, D]
                for kl in range(SLAB // 128):
                    kc = (s0 // 128) + kl
                    for dh in range(2):
                        ps = p_ps.tile([128, 512], F32, tag="ps")
                        for di in range(NDC):
                            nc.tensor.matmul(
                                ps[:, :],
                                xts[di][:, 128 * kl:128 * (kl + 1)],
                                wv[di][:, 512 * dh:512 * (dh + 1)],
                                start=(di == 0), stop=(di == NDC - 1),
                            )
                        nc.vector.tensor_copy(vsb[kc][:, 512 * dh:512 * (dh + 1)], ps[:, :])
                # kbias[k] = K @ (bq/sqrt(D)) for this slab
                for kl in range(SLAB // 128):
                    kc = (s0 // 128) + kl
                    ps1 = p_ps1.tile([128, 1], F32, tag="ps1")
                    for do in range(NDC):
                        nc.tensor.matmul(
                            ps1[:, :],
                            kts[do][:, 128 * kl:128 * (kl + 1)],
                            bqT[:, do:do + 1],
                            start=(do == 0), stop=(do == NDC - 1),
                        )
                    nc.vector.tensor_copy(kbias[:, kc:kc + 1], ps1[:, :])
                # ST -> exp -> PT
                for kl in range(SLAB // 128):
                    kc = (s0 // 128) + kl
                    for qh in range(2):
                        ps = p_ps.tile([128, 512], F32, tag="ps")
                        for do in range(NDC):
                            nc.tensor.matmul(
                                ps[:, :],
                                kts[do][:, 128 * kl:128 * (kl + 1)],
                                qt[do][:, 512 * qh:512 * (qh + 1)],
                                start=(do == 0), stop=(do == NDC - 1),
                            )
                        nc.scalar.activation(
                            out=pt[kc][:, 512 * qh:512 * (qh + 1)], in_=ps[:, :],
                            func=mybir.ActivationFunctionType.Exp,
                            bias=kbias[:, kc:kc + 1], scale=SCALE,
                        )

            # ---- AT[d,q] = V^T @ PT ; denom[q] = ones^T @ PT ----
            with tc.tile_pool(name="atp", bufs=NDC) as p_at:
                at = [p_at.tile([128, Q], BF16, tag="at", name=f"at{i}") for i in range(NDC)]
                for dc in range(NDC):
                    for qh in range(2):
                        ps = p_ps.tile([128, 512], F32, tag="ps")
                        for kc in range(NKC):
                            nc.tensor.matmul(
                                ps[:, :],
                                vsb[kc][:, 128 * dc:128 * (dc + 1)],
                                pt[kc][:, 512 * qh:512 * (qh + 1)],
                                start=(kc == 0), stop=(kc == NKC - 1),
                            )
                        nc.vector.tensor_copy(at[dc][:, 512 * qh:512 * (qh + 1)], ps[:, :])
                for qp in range(8):
                    ps1 = p_ps1.tile([128, 1], F32, tag="ps1")
                    for kc in range(NKC):
                        nc.tensor.matmul(
                            ps1[:, :],
                            pt[kc][:, 128 * qp:128 * (qp + 1)],
                            ones[:, :],
                            start=(kc == 0), stop=(kc == NKC - 1),
                        )
                    nc.vector.reciprocal(recip[:, qp:qp + 1], ps1[:, :])

                # ---- O = AT^T @ wo ; /denom ; +xq_aug ; LayerNorm ----
                with (
                    tc.tile_pool(name="wop", bufs=NDC) as p_wo,
                    tc.tile_pool(name="xqp", bufs=3) as p_xq,
                    tc.tile_pool(name="vout", bufs=3) as p_vo,
                    tc.tile_pool(name="lnst", bufs=4) as p_ln,
                ):
                    wo = [p_wo.tile([128, D], BF16, tag="wo", name=f"wo{i}") for i in range(NDC)]
                    for dc in range(NDC):
                        nc.gpsimd.dma_start(out=wo[dc][:, :], in_=wo_d[128 * dc:128 * (dc + 1), :])
                    gam = p_small.tile([128, D], F32)
                    nc.gpsimd.dma_start(out=gam[:, :], in_=_bcast_ap(gamma_d[:]))
                    bet = p_small.tile([128, D], F32)
                    nc.gpsimd.dma_start(out=bet[:, :], in_=_bcast_ap(beta_d[:]))

                    for qp in range(8):
                        v = p_vo.tile([128, D], F32, tag="v")
                        sqs = p_vo.tile([128, D], F32, tag="sqs")
                        xqt_ = p_xq.tile([128, D], F32, tag="xq")
                        nc.sync.dma_start(
                            out=xqt_[:, :], in_=xq[128 * qp:128 * (qp + 1), :]
                        )
                        st = p_ln.tile([128, 4], F32, tag="st")
                        for eh in range(2):
                            ps = p_ps.tile([128, 512], F32, tag="ps")
                            for dc in range(NDC):
                                nc.tensor.matmul(
                                    ps[:, :],
                                    at[dc][:, 128 * qp:128 * (qp + 1)],
                                    wo[dc][:, 512 * eh:512 * (eh + 1)],
                                    start=(dc == 0), stop=(dc == NDC - 1),
                                )
                            # v_half = O*1/denom + xq; accum = row-sum of v_half
                            nc.vector.scalar_tensor_tensor(
                                out=v[:, 512 * eh:512 * (eh + 1)], in0=ps[:, :],
                                scalar=recip[:, qp:qp + 1],
                                in1=xqt_[:, 512 * eh:512 * (eh + 1)],
                                op0=mybir.AluOpType.mult, op1=mybir.AluOpType.add,
                                accum_out=st[:, eh:eh + 1],
                            )
                        # E[v^2] on ACT (Square + free accum); mean on DVE/ACT
                        nc.scalar.activation(
                            out=sqs[:, :], in_=v[:, :],
                            func=mybir.ActivationFunctionType.Square,
                            accum_out=st[:, 2:3],
                        )
                        nc.vector.tensor_add(st[:, 0:1], st[:, 0:1], st[:, 1:2])
                        nc.scalar.mul(st[:, 0:1], st[:, 0:1], 1.0 / D)     # mean
                        nc.scalar.mul(st[:, 2:3], st[:, 2:3], 1.0 / D)     # E[v^2]
                        nc.vector.tensor_mul(st[:, 1:2], st[:, 0:1], st[:, 0:1])
                        nc.vector.tensor_sub(st[:, 2:3], st[:, 2:3], st[:, 1:2])  # var
                        nc.scalar.activation(
                            out=st[:, 2:3], in_=st[:, 2:3],
                            func=mybir.ActivationFunctionType.Sqrt,
                            bias=eps_t[:, :],
                        )
                        nc.vector.reciprocal(st[:, 2:3], st[:, 2:3])       # rstd
                        # out = ((v - mean)*gamma)*rstd + beta  (2 fused DVE ops)
                        nc.vector.scalar_tensor_tensor(
                            out=v[:, :], in0=v[:, :], scalar=st[:, 0:1],
                            in1=gam[:, :],
                            op0=mybir.AluOpType.subtract, op1=mybir.AluOpType.mult,
                        )
                        nc.vector.scalar_tensor_tensor(
                            out=v[:, :], in0=v[:, :], scalar=st[:, 2:3],
                            in1=bet[:, :],
                            op0=mybir.AluOpType.mult, op1=mybir.AluOpType.add,
                        )
                        nc.sync.dma_start(out=out_d[128 * qp:128 * (qp + 1), :], in_=v[:, :])
    nc.compile()
    return nc


_NC_CACHE = None


def kernel(**inputs) -> np.ndarray:
    from concourse.bass_utils import run_bass_kernel_spmd

    global _NC_CACHE
    x = np.asarray(inputs["inputs"], np.float32)
    wq = np.asarray(inputs["wq"], np.float32)
    wk = np.asarray(inputs["wk"], np.float32)
    wv = np.asarray(inputs["wv"], np.float32)
    wo = np.asarray(inputs["wo"], np.float32)
    bq = np.asarray(inputs["bq"], np.float32)
    bk = np.asarray(inputs["bk"], np.float32)
    bv = np.asarray(inputs["bv"], np.float32)
    bo = np.asarray(inputs["bo"], np.float32)
    gamma = np.asarray(inputs["gamma"], np.float32)
    beta = np.asarray(inputs["beta"], np.float32)

    bf = lambda a: np.ascontiguousarray(a).astype(ml_dtypes.bfloat16)
    bo_eff = bo + bv @ wo  # exact: rows of softmax sum to 1
    shared = {
        "wq": bf(wq), "wk": bf(wk), "wv": bf(wv), "wo": bf(wo),
        "bqT": bf((bq * SCALE).reshape(NDC, 128).T),
        "bkT": np.ascontiguousarray(bk.reshape(NDC, 128).T),
        "gamma": gamma, "beta": beta,
    }
    in_maps = []
    for c in range(8):
        b, qh = c // 2, c % 2
        xtb = np.ascontiguousarray(x[b].T)
        in_maps.append({
            **shared,
            "xt": bf(xtb),
            "xqt": bf(xtb[:, Q * qh:Q * (qh + 1)]),
            "xq": np.ascontiguousarray(x[b, Q * qh:Q * (qh + 1), :]) + bo_eff[None, :],
        })

    if _NC_CACHE is None:
        _NC_CACHE = build_nc()
    res = run_bass_kernel_spmd(_NC_CACHE, in_maps, core_ids=list(range(8)))
    out = np.empty((B, S, D), np.float32)
    for c in range(8):
        b, qh = c // 2, c % 2
        out[b, Q * qh:Q * (qh + 1), :] = res.results[c]["out"]
    return out
